# revision 12
# baseline (speedup 1.0000x reference)
# Trainium2 Bass kernel: Llama-style attention block (GQA + RoPE + causal),
# tensor-parallel across heads on 8 NeuronCores.
#
# Full-shape contract: kernel(**inputs) takes the unsharded numpy inputs and
# returns the full [B, S, HID] float32 output.
#
# Sharding strategy (per core i of 8):
#   - 4 query heads (rows i*512:(i+1)*512 of Wq) + 1 kv head (rows i*128.. of Wk/Wv)
#   - Wo is sharded row-wise (its columns i*512:(i+1)*512); each core emits a
#     partial [B,S,HID] product (bf16) which the host sums at gather time.
# All weights/activations are pre-transposed and pre-tiled on the host so the
# device kernel needs zero on-chip transposes of activations:
#   matmul(out[M,N], lhsT[K,M], rhs[K,N]) contracts over the partition dim K.
#
# Schedule (deferred-AV): the AV matmuls for head h are interleaved one-per-kt
# into head h+1's score stream, so by the time an AV issues, its exp input was
# produced a full head earlier and the PE never waits on the scalar engine.
# The softmax normalization is a 3-stage pipeline riding fixed slots of the
# next stream: sum(h) @ stream h+1 slot 1, bcast+recip(h) @ slot 3, apply(h)
# at stream h+1's end (right after av(h)'s chain stops).  The last head of
# each chunk flushes its AV + norm work into the next chunk's projection sweep.
#
# The projection sweep runs as 5 sequential passes (K, q0..q3), each followed
# immediately by its RoPE eviction on the DVE: psum banks free progressively
# during the sweep (instead of all five at the end), so the next stream's
# score tiles never wait on the psum ring, and all RoPE work overlaps the
# sweep itself instead of stalling the boundary.
#
# Startup DMA is just-in-time: wk/ht interleaved per-kt on the sync queue,
# chunk-0 cos/sin slices + wq early on the scalar queue, wv before the bulk
# trig/wo loads.

import os
import sys
from contextlib import ExitStack

for _p in ("/opt/trn_rl_repo", "/root/.axon_site/_ro/trn_rl_repo"):
    if os.path.isdir(_p) and _p not in sys.path:
        sys.path.append(_p)

import ml_dtypes
import numpy as np

import concourse.bass as bass
import concourse.mybir as mybir
import concourse.tile as tile
from concourse import bacc
from concourse.bass_utils import run_bass_kernel_spmd

BF16 = mybir.dt.bfloat16
F32 = mybir.dt.float32
NEG = -1.0e9
N_CORES = 8


def build_core_kernel(B, S, HID, QH=4, D=128, QCH=512):
    """SPMD per-core program. QH query heads + 1 kv head per core.

    DRAM parameter layouts (host pre-tiles everything):
      ht   [B, KT, TC, 128, QCH]  bf16  hidden^T tiles
      trig [B, 2, D, S]           bf16  cos^T / sin^T
      wq   [KT, 128, QH*D]        bf16  Wq_core^T tiles
      wk   [KT, 128, D]           bf16
      wv   [KT, 128, D]           bf16
      wo   [QH, D, HID]           bf16  Wo_core^T rows blocked per head
      mask [128, QCH]             f32   additive causal triangle
      ones [128, 1]               bf16  partition-sum lhsT
      onesr [1, 128]              bf16  broadcast lhsT
      eye  [128, 128]             bf16
      out  [B, TT, HC, 128, QCH]  bf16  partial output tiles
    """
    FS = QH * D          # per-core feature slice of the qkv/attn space
    KT = HID // 128      # contraction tiles for projections
    TC = S // QCH        # 512-token chunks
    TT = S // 128        # 128-token tiles
    TPC = QCH // 128     # token tiles per chunk
    HC = HID // QCH      # output hid chunks
    KPQ = QCH // 128     # k-tiles per q-chunk (diagonal band width)
    HALF = D // 2
    SC = float(1.0 / np.sqrt(D))
    EXP = mybir.ActivationFunctionType.Exp

    # Bacc (not plain Bass): its compile pipeline splits multi-sem waits into
    # EventSemaphore instructions — the DMA DIRECT2D struct has one wait slot.
    nc = bacc.Bacc(None)
    ht = nc.declare_dram_parameter("ht", [B, KT, TC, 128, QCH], BF16, isOutput=False)
    trig = nc.declare_dram_parameter("trig", [B, 2, D, S], BF16, isOutput=False)
    wq = nc.declare_dram_parameter("wq", [KT, 128, FS], BF16, isOutput=False)
    wk = nc.declare_dram_parameter("wk", [KT, 128, D], BF16, isOutput=False)
    wv = nc.declare_dram_parameter("wv", [KT, 128, D], BF16, isOutput=False)
    wo = nc.declare_dram_parameter("wo", [QH, D, HID], BF16, isOutput=False)
    mask = nc.declare_dram_parameter("mask", [128, QCH], F32, isOutput=False)
    ones = nc.declare_dram_parameter("ones", [128, 1], BF16, isOutput=False)
    onesr = nc.declare_dram_parameter("onesr", [1, 128], BF16, isOutput=False)
    eye = nc.declare_dram_parameter("eye", [128, 128], BF16, isOutput=False)
    out = nc.declare_dram_parameter("out", [B, TT, HC, 128, QCH], BF16, isOutput=True)

    with ExitStack() as ctx:
        tc = ctx.enter_context(tile.TileContext(nc))
        pool = lambda name, bufs, space=None: ctx.enter_context(
            tc.tile_pool(name=name, bufs=bufs, **({"space": space} if space else {}))
        )
        p_w = pool("p_w", 1)          # weights + constants, loaded once
        p_ht = pool("p_ht", 44)       # streamed hidden^T tiles (chunk + 12 prefetch)
        p_qt = pool("p_qt", 8)        # per-chunk per-head Q^T [D, QCH] bf16
        p_kt = pool("p_kt", 1)        # K^T [D, S] bf16 (persistent per batch)
        p_vtt = pool("p_vtt", 3)      # V^T staging before transpose
        p_vt = pool("p_vt", S // 128 + 4)  # V tiles [128 tok, D]
        p_at = pool("p_at", 8)        # per-chunk per-head attn^T [D, QCH] bf16
        p_exp = pool("p_exp", 20)     # exp(score) tiles bf16 (a full head + next)
        p_rt = pool("p_rt", 1)        # rope temps bf16
        p_acc = pool("p_acc", 2)      # softmax-sum accumulators bf16
        p_sums = pool("p_sums", 2)    # softmax sums row bf16
        p_rbc = pool("p_rbc", 2)      # broadcast reciprocal f32
        p_ost = pool("p_ost", 4)      # output staging bf16
        p_pp = pool("p_pp", 6, "PSUM")    # kps/qps/st/pw/sum/bc/transpose ring
        p_av = pool("p_av", 2, "PSUM")    # attn-v accumulators (+proj V)

        wq_sb = p_w.tile([128, KT, FS], BF16, name="wq_sb")
        wk_sb = p_w.tile([128, KT, D], BF16, name="wk_sb")
        wv_sb = p_w.tile([128, KT, D], BF16, name="wv_sb")
        wo_sb = p_w.tile([128, QH, HID], BF16, name="wo_sb")
        mask_sb = p_w.tile([128, QCH], F32, name="mask_sb")
        ones_sb = p_w.tile([128, 1], BF16, name="ones_sb")
        onesr_sb = p_w.tile([1, 128], BF16, name="onesr_sb")
        eye_sb = p_w.tile([128, 128], BF16, name="eye_sb")
        cos_sb = p_w.tile([D, B, S], BF16, name="cos_sb")
        sin_sb = p_w.tile([D, B, S], BF16, name="sin_sb")
        # Startup order (scalar queue): tiny consts + chunk-0 trig slices, the
        # wq tiles the first sweep consumes, then wv (needed at sweep2), the
        # remaining trig slices, then wo (first outproj at ~150us).  wk and
        # the chunk-0 ht tiles ride the sync queue interleaved per-kt inside
        # the first sweep pass so the PE is fed from ~2us.
        nc.scalar.dma_start(out=mask_sb[:, :], in_=mask[:, :])
        nc.scalar.dma_start(out=eye_sb[:, :], in_=eye[:, :])
        nc.scalar.dma_start(out=cos_sb[:, 0, 0:QCH], in_=trig[0, 0][:, 0:QCH])
        nc.scalar.dma_start(out=sin_sb[:, 0, 0:QCH], in_=trig[0, 1][:, 0:QCH])
        for kt in range(KT):
            nc.scalar.dma_start(out=wq_sb[:, kt, :], in_=wq[kt])
        nc.scalar.dma_start(out=ones_sb[:, :], in_=ones[:, :])
        nc.scalar.dma_start(out=onesr_sb[:, :], in_=onesr[:, :])
        for kt in range(KT):
            nc.scalar.dma_start(out=wv_sb[:, kt, :], in_=wv[kt])
        for b in range(B):
            for tci in range(TC):
                if b == 0 and tci == 0:
                    continue
                sl = slice(tci * QCH, (tci + 1) * QCH)
                nc.scalar.dma_start(out=cos_sb[:, b, sl], in_=trig[b, 0][:, sl])
                nc.scalar.dma_start(out=sin_sb[:, b, sl], in_=trig[b, 1][:, sl])
        for f in range(QH):
            nc.scalar.dma_start(out=wo_sb[:, f, :], in_=wo[f])

        def rope_evict(b, dst, ps, tci):
            # dst[:, :] = ps * cos + rotate_half(ps) * sin  (write bf16)
            # bf16 temporaries: DVE runs 2 elems/cycle for 16-bit vs 1 for f32.
            sl = slice(tci * QCH, (tci + 1) * QCH)
            cs = cos_sb[:, b, sl]
            sn = sin_sb[:, b, sl]
            t1 = p_rt.tile([128, QCH], BF16, name="rt1", tag="rt1")
            t2 = p_rt.tile([128, QCH], BF16, name="rt2", tag="rt2")
            nc.vector.tensor_mul(t1[:, :], ps[:, :], cs)
            nc.vector.tensor_mul(t2[0:HALF, :], ps[HALF:D, :], sn[0:HALF, :])
            nc.vector.tensor_mul(t2[HALF:D, :], ps[0:HALF, :], sn[HALF:D, :])
            nc.vector.tensor_sub(dst[0:HALF, :], t1[0:HALF, :], t2[0:HALF, :])
            nc.vector.tensor_add(dst[HALF:D, :], t1[HALF:D, :], t2[HALF:D, :])

        # Output projection emitters for one chunk (32 (tt,hc) groups); popped
        # one-per-kt into later streams once all four at-tiles are applied.
        def emit_op_group(g):
            bb, tt, tt4, hc, atp = g
            csl = slice(tt4 * 128, (tt4 + 1) * 128)
            pw = p_pp.tile([128, QCH], F32, name="ps_wo", tag="pp")
            for f in range(QH):
                nc.tensor.matmul(pw[:, :], atp[f][:, csl],
                                 wo_sb[:, f, hc * QCH:(hc + 1) * QCH],
                                 start=(f == 0), stop=(f == QH - 1))
            o = p_ost.tile([128, QCH], BF16, name="o_t", tag="ost")
            # evictions mostly on vector: each o-copy on the scalar queue sits
            # ahead of later exps and stalls the score psum ring, so scalar
            # (which must stay clear for the exp stream) takes only every 4th.
            # gpsimd cannot read PSUM, so it can't help here.
            if hc % 4 == 3:
                nc.scalar.copy(o[:, :], pw[:, :])
            else:
                nc.vector.tensor_copy(o[:, :], pw[:, :])
            nc.sync.dma_start(out=out[bb, tt, hc], in_=o[:, :])

        def make_op_groups(bb, tcip, atp):
            return [(bb, tcip * TPC + tt4, tt4, hc, atp)
                    for tt4 in range(TPC) for hc in range(HC)]

        # ---- norm pipeline + deferred-AV state (crosses streams/chunks) ----
        # P1: (acc, av, at_dst) awaiting sum+bcast in the current stream.
        # P2: (rb, av, at_dst) awaiting apply at the next stream's slot 0.
        # prev_av: the last finished head's AV work, flushed one-per-slot into
        #          the next stream (or the next chunk's projection sweep).
        st_ = {"P1": None, "P2": None, "prev_av": None}

        def emit_sum(P1):
            acc, av, at_dst = P1
            sm = p_pp.tile([1, QCH], F32, name="ps_sm", tag="pp")
            nc.tensor.matmul(sm[:, :], ones_sb[:, :], acc[:, :],
                             start=True, stop=True)
            sms = p_sums.tile([1, QCH], BF16, name="sms", tag="sums")
            nc.scalar.copy(sms[:, :], sm[:, :])
            return (sms, av, at_dst)

        def emit_bc(P1s):
            sms, av, at_dst = P1s
            pb = p_pp.tile([128, QCH], F32, name="ps_pb", tag="pp")
            nc.tensor.matmul(pb[:, :], onesr_sb[:, :], sms[:, :],
                             start=True, stop=True)
            rb = p_rbc.tile([128, QCH], F32, name="rb", tag="rb")
            nc.vector.reciprocal_approx_fast(rb[:, :], pb[:, :])
            return (rb, av, at_dst)

        def emit_apply(P2):
            rb, av, at_dst = P2
            nc.vector.tensor_mul(at_dst[:, :], av[:, :], rb[:, :])

        def emit_av_step(pa, kt):
            e, lo, n = pa["es"][kt]
            nc.tensor.matmul(pa["av"][:, lo:QCH], pa["vb"][kt][:, :], e[:, 0:n],
                             start=(kt == 0), stop=(kt == pa["nk"] - 1))

        def stream_slot(kt, oq):
            # Fixed-slot work interleaved into a score stream: deferred AV
            # one-per-kt, norm stages at slots 1/3, outproj pops from kt>=3.
            # (The pending apply runs at stream END — right after the AV chain
            # it reads stops, and before any later psum alloc can rebind it.)
            pa = st_["prev_av"]
            if pa is not None and kt < pa["nk"]:
                emit_av_step(pa, kt)
                if kt == pa["nk"] - 1:
                    st_["prev_av"] = None
            if kt == 1 and st_["P1"] is not None:
                st_["P1"] = emit_sum(st_["P1"])
            if kt == 3 and st_["P1"] is not None:
                st_["P2"] = emit_bc(st_["P1"])
                st_["P1"] = None
            if kt >= 3 and oq:
                emit_op_group(oq.pop(0))

        def sweep1_slot(g):
            # Slot work inside the 5x32 projection sweep: the previous
            # chunk's last-head AV flush (g 0..nk'-1), then its norm stages
            # late enough that kps' psum slot has been freed by its RoPE
            # eviction (sum@g40, bc@g46) and its apply once its AV chain
            # stopped (g52).  No outproj pops: the pp ring is full of qps.
            pa = st_["prev_av"]
            if pa is not None and g < pa["nk"]:
                emit_av_step(pa, g)
                if g == pa["nk"] - 1:
                    st_["prev_av"] = None
            if g == 40 and st_["P1"] is not None:
                st_["P1"] = emit_sum(st_["P1"])
            if g == 46 and st_["P1"] is not None:
                st_["P2"] = emit_bc(st_["P1"])
                st_["P1"] = None
            if g == 52 and st_["P2"] is not None:
                emit_apply(st_["P2"])
                st_["P2"] = None

        prev_op = None    # (b, tci, at_c) of the chunk awaiting outproj

        for b in range(B):
            vb = []       # V tiles [128 tok, D], global k-tile index
            ktb = p_kt.tile([D, S], BF16, name="ktb", tag="kt")
            for tci in range(TC):
                first = (b == 0 and tci == 0)
                oq = make_op_groups(*prev_op) if prev_op else []
                ktb_c = ktb[:, tci * QCH:(tci + 1) * QCH]
                qtb_c = [p_qt.tile([D, QCH], BF16, name=f"qtc{f}", tag="qt")
                         for f in range(QH)]
                # ---- sweep 1: five passes (K, q0..q3) over the kt tiles;
                #      each pass's RoPE eviction trails it on the DVE ----
                hts = []
                for p in range(1 + QH):
                    ps = p_pp.tile([128, QCH], F32, name=f"ps_p{p}", tag="pp")
                    for kt in range(KT):
                        if p == 0:
                            if first:
                                nc.sync.dma_start(out=wk_sb[:, kt, :], in_=wk[kt])
                            t = p_ht.tile([128, QCH], BF16, name="ht_t", tag="ht")
                            nc.sync.dma_start(out=t[:, :], in_=ht[b, kt, tci])
                            hts.append(t)
                        w_sl = (wk_sb[:, kt, :] if p == 0 else
                                wq_sb[:, kt, (p - 1) * D:p * D])
                        nc.tensor.matmul(ps[:, :], w_sl, hts[kt][:, :],
                                         start=(kt == 0), stop=(kt == KT - 1))
                        sweep1_slot(p * KT + kt)
                    if p == 0:
                        rope_evict(b, ktb_c, ps, tci)
                    else:
                        rope_evict(b, qtb_c[p - 1], ps, tci)
                # ---- sweep 2: V^T accumulation, then PE-transpose ----
                vps = p_av.tile([128, QCH], F32, name="ps_v", tag="av")
                for kt in range(KT):
                    nc.tensor.matmul(vps[:, :], wv_sb[:, kt, :], hts[kt][:, :],
                                     start=(kt == 0), stop=(kt == KT - 1))
                vtt = p_vtt.tile([128, QCH], BF16, name="vtt", tag="vtt")
                nc.vector.tensor_copy(vtt[:, :], vps[:, :])
                for sub in range(TPC):
                    pt = p_pp.tile([128, 128], BF16, name="ps_vt", tag="pp")
                    nc.tensor.transpose(pt[:, :], vtt[:, sub * 128:(sub + 1) * 128],
                                        eye_sb[:, :])
                    v = p_vt.tile([128, D], BF16, name="v_t", tag="vt")
                    nc.scalar.copy(v[:, :], pt[:, :])
                    vb.append(v)

                # ---- attention streams: scores for head h interleaved with
                #      the deferred AV of head h-1, norm slots, outproj pops ----
                at_c = [p_at.tile([D, QCH], BF16, name=f"atc{f}", tag="at")
                        for f in range(QH)]
                nk = KPQ * (tci + 1)
                for h in range(QH):
                    av = p_av.tile([128, QCH], F32, name="ps_av", tag="av")
                    acc = p_acc.tile([128, QCH], BF16, name="acc", tag="acc")
                    es = [None] * nk
                    for kt in range(nk):
                        j = kt - KPQ * tci          # >=0 inside diagonal band
                        lo = max(j, 0) * 128        # first live column
                        n = QCH - lo
                        st = p_pp.tile([128, QCH], F32, name="ps_st", tag="pp")
                        nc.tensor.matmul(
                            st[:, 0:n], ktb[:, kt * 128:(kt + 1) * 128],
                            qtb_c[h][:, lo:QCH], start=True, stop=True)
                        if j >= 0:  # diagonal tile: triangular mask on 1st 128
                            nc.vector.tensor_add(st[:, 0:128], st[:, 0:128],
                                                 mask_sb[:, 0:128])
                        e = p_exp.tile([128, QCH], BF16, name="e_t", tag="exp")
                        nc.scalar.activation(e[:, 0:n], st[:, 0:n], EXP, scale=SC)
                        # softmax-sum accumulation on gpsimd (SBUF-only data):
                        # the only engine with idle capacity, and it frees the
                        # DVE queue for rope/apply/eviction work.
                        if kt == 0:
                            nc.gpsimd.tensor_copy(acc[:, :], e[:, :])
                        else:
                            nc.gpsimd.tensor_add(acc[:, lo:QCH], acc[:, lo:QCH],
                                                 e[:, 0:n])
                        es[kt] = (e, lo, n)
                        stream_slot(kt, oq)
                    assert st_["prev_av"] is None
                    assert st_["P1"] is None
                    # apply the head-before-last now: its AV chain stopped in
                    # this stream, and deferring further would let a later
                    # p_av allocation rebind its psum slot.
                    if st_["P2"] is not None:
                        emit_apply(st_["P2"])
                        st_["P2"] = None
                    st_["prev_av"] = {"av": av, "es": es, "vb": list(vb), "nk": nk}
                    st_["P1"] = (acc, av, at_c[h])
                # leftover outproj groups of the previous chunk
                for g in oq:
                    emit_op_group(g)
                prev_op = (b, tci, at_c)

        # ---- tail: flush the last head's AV + normalization + final outproj ----
        if st_["P2"] is not None:
            emit_apply(st_["P2"])   # at of head 2 of the last chunk
            st_["P2"] = None
        pa = st_["prev_av"]
        for kt in range(pa["nk"]):
            emit_av_step(pa, kt)
            if kt == 1 and st_["P1"] is not None:
                st_["P1"] = emit_sum(st_["P1"])
            if kt == 3 and st_["P1"] is not None:
                st_["P2"] = emit_bc(st_["P1"])
                st_["P1"] = None
        st_["prev_av"] = None
        emit_apply(st_["P2"])       # at of head 3
        st_["P2"] = None
        for g in make_op_groups(*prev_op):
            emit_op_group(g)
    nc.finalize()  # Bacc: runs compile() (reg alloc, wait splitting) + freeze
    return nc


def shard_inputs(hidden_states, cos, sin, Wq, Wk, Wv, Wo, n_cores=N_CORES,
                 QH=4, D=128, QCH=512):
    """Host-side prep: transpose/tile/bf16-round everything per core."""
    bf16 = ml_dtypes.bfloat16
    B, S, HID = hidden_states.shape
    FS = QH * D
    KT = HID // 128
    TC = S // QCH

    hT = hidden_states.astype(bf16).transpose(0, 2, 1)           # [B, HID, S]
    ht_t = np.ascontiguousarray(
        hT.reshape(B, KT, 128, TC, QCH).transpose(0, 1, 3, 2, 4))
    trig = np.ascontiguousarray(np.stack(
        [cos.transpose(0, 2, 1), sin.transpose(0, 2, 1)], axis=1)
    ).astype(bf16)

    kk = np.arange(128)[:, None]
    cc = np.arange(QCH)[None, :]
    maskv = np.where(cc < kk, np.float32(NEG), np.float32(0.0))
    onesv = np.ones((128, 1), dtype=bf16)
    onesrv = np.ones((1, 128), dtype=bf16)
    eyev = np.eye(128, dtype=bf16)

    in_maps = []
    for i in range(n_cores):
        wq_i = Wq[i * FS:(i + 1) * FS, :].T.astype(bf16).reshape(KT, 128, FS)
        wk_i = Wk[i * D:(i + 1) * D, :].T.astype(bf16).reshape(KT, 128, D)
        wv_i = Wv[i * D:(i + 1) * D, :].T.astype(bf16).reshape(KT, 128, D)
        wo_i = Wo[:, i * FS:(i + 1) * FS].T.astype(bf16).reshape(QH, D, HID)
        in_maps.append(dict(ht=ht_t, trig=trig, wq=wq_i, wk=wk_i, wv=wv_i,
                            wo=wo_i, mask=maskv, ones=onesv, onesr=onesrv,
                            eye=eyev))
    return in_maps


_NC_CACHE = {}


def kernel(hidden_states, cos, sin, Wq, Wk, Wv, Wo, _trace=False):
    hidden_states = np.asarray(hidden_states)
    cos = np.asarray(cos)
    sin = np.asarray(sin)
    Wq, Wk, Wv, Wo = (np.asarray(a) for a in (Wq, Wk, Wv, Wo))
    B, S, HID = hidden_states.shape

    key = (B, S, HID)
    nc = _NC_CACHE.get(key)
    if nc is None:
        nc = _NC_CACHE[key] = build_core_kernel(B, S, HID)

    in_maps = shard_inputs(hidden_states, cos, sin, Wq, Wk, Wv, Wo)
    res = run_bass_kernel_spmd(nc, in_maps, core_ids=list(range(N_CORES)),
                               trace=_trace)
    kernel._last_results = res

    acc = res.results[0]["out"].astype(np.float32)
    for r in res.results[1:]:
        acc += r["out"].astype(np.float32)
    # [B, TT, HC, 128, QCH] -> [B, S, HID]
    TT = S // 128
    HC = HID // 512
    full = acc.transpose(0, 1, 3, 2, 4).reshape(B, S, HID)
    return np.ascontiguousarray(full)


# revision 23
# speedup vs baseline: 1.1025x; 1.1025x over previous
# Trainium2 Bass kernel: Llama-style attention block (GQA + RoPE + causal),
# tensor-parallel across heads on 8 NeuronCores.
#
# Full-shape contract: kernel(**inputs) takes the unsharded numpy inputs and
# returns the full [B, S, HID] float32 output.
#
# Sharding strategy (per core i of 8):
#   - 4 query heads (rows i*512:(i+1)*512 of Wq) + 1 kv head (rows i*128.. of Wk/Wv)
#   - Wo is sharded row-wise (its columns i*512:(i+1)*512); each core emits a
#     partial [B,S,HID] product (bf16) which the host sums at gather time.
# All weights/activations are pre-transposed and pre-tiled on the host so the
# device kernel needs zero on-chip transposes of activations:
#   matmul(out[M,N], lhsT[K,M], rhs[K,N]) contracts over the partition dim K.
#
# Schedule (deferred-AV): the AV matmuls for head h are interleaved one-per-kt
# into head h+1's score stream, so by the time an AV issues, its exp input was
# produced a full head earlier and the PE never waits on the scalar engine.
# The softmax normalization is a 3-stage pipeline riding fixed slots of the
# next stream: sum(h) @ stream h+1 slot 1, bcast+recip(h) @ slot 3, apply(h)
# at stream h+1's end (right after av(h)'s chain stops).  The last head of
# each chunk flushes its AV + norm work into the next chunk's projection sweep.
#
# The projection sweep runs as 5 sequential passes (K, q0..q3), each followed
# immediately by its RoPE eviction on the DVE: psum banks free progressively
# during the sweep (instead of all five at the end), so the next stream's
# score tiles never wait on the psum ring, and all RoPE work overlaps the
# sweep itself instead of stalling the boundary.
#
# Startup DMA is just-in-time: wk/ht interleaved per-kt on the sync queue,
# chunk-0 cos/sin slices + wq early on the scalar queue, wv before the bulk
# trig/wo loads.

import os
import sys
from contextlib import ExitStack

for _p in ("/opt/trn_rl_repo", "/root/.axon_site/_ro/trn_rl_repo"):
    if os.path.isdir(_p) and _p not in sys.path:
        sys.path.append(_p)

import ml_dtypes
import numpy as np

import concourse.bass as bass
import concourse.mybir as mybir
import concourse.tile as tile
from concourse import bacc
from concourse.bass_utils import run_bass_kernel_spmd

BF16 = mybir.dt.bfloat16
F32 = mybir.dt.float32
NEG = -1.0e9
N_CORES = 8


def build_core_kernel(B, S, HID, QH=4, D=128, QCH=512):
    """SPMD per-core program. QH query heads + 1 kv head per core.

    DRAM parameter layouts (host pre-tiles everything):
      ht   [B, KT, TC, 128, QCH]  bf16  hidden^T tiles
      trig [B, 2, D, S]           bf16  cos^T / sin^T
      wq   [KT, 128, QH*D]        bf16  Wq_core^T tiles
      wk   [KT, 128, D]           bf16
      wv   [KT, 128, D]           bf16
      wo   [QH, D, HID]           bf16  Wo_core^T rows blocked per head
      mask [128, QCH]             f32   additive causal triangle
      ones [128, 1]               bf16  partition-sum lhsT
      onesr [1, 128]              bf16  broadcast lhsT
      eye  [128, 128]             bf16
      out  [B, TT, HC, 128, QCH]  bf16  partial output tiles
    """
    FS = QH * D          # per-core feature slice of the qkv/attn space
    KT = HID // 128      # contraction tiles for projections
    TC = S // QCH        # 512-token chunks
    TT = S // 128        # 128-token tiles
    TPC = QCH // 128     # token tiles per chunk
    HC = HID // QCH      # output hid chunks
    KPQ = QCH // 128     # k-tiles per q-chunk (diagonal band width)
    HALF = D // 2
    SC = float(1.0 / np.sqrt(D))
    EXP = mybir.ActivationFunctionType.Exp

    # Bacc (not plain Bass): its compile pipeline splits multi-sem waits into
    # EventSemaphore instructions — the DMA DIRECT2D struct has one wait slot.
    nc = bacc.Bacc(None)
    ht = nc.declare_dram_parameter("ht", [B, KT, TC, 128, QCH], BF16, isOutput=False)
    trig = nc.declare_dram_parameter("trig", [B, 2, D, S], BF16, isOutput=False)
    wq = nc.declare_dram_parameter("wq", [KT, 128, FS], BF16, isOutput=False)
    wk = nc.declare_dram_parameter("wk", [KT, 128, D], BF16, isOutput=False)
    wv = nc.declare_dram_parameter("wv", [KT, 128, D], BF16, isOutput=False)
    wo = nc.declare_dram_parameter("wo", [QH, D, HID], BF16, isOutput=False)
    mask = nc.declare_dram_parameter("mask", [128, QCH], F32, isOutput=False)
    onesq = nc.declare_dram_parameter("onesq", [128, 128], BF16, isOutput=False)
    eye = nc.declare_dram_parameter("eye", [128, 128], BF16, isOutput=False)
    out = nc.declare_dram_parameter("out", [B, TT, HC, 128, QCH], BF16, isOutput=True)

    with ExitStack() as ctx:
        tc = ctx.enter_context(tile.TileContext(nc))
        pool = lambda name, bufs, space=None: ctx.enter_context(
            tc.tile_pool(name=name, bufs=bufs, **({"space": space} if space else {}))
        )
        p_w = pool("p_w", 1)          # weights + constants, loaded once
        p_ht = pool("p_ht", 44)       # streamed hidden^T tiles (chunk + 12 prefetch)
        p_qt = pool("p_qt", 8)        # per-chunk per-head Q^T [D, QCH] bf16
        p_kt = pool("p_kt", 1)        # K^T [D, S] bf16 (persistent per batch)
        p_vtt = pool("p_vtt", 3)      # V^T staging before transpose
        p_vt = pool("p_vt", S // 128 + 4)  # V tiles [128 tok, D]
        p_at = pool("p_at", 8)        # per-chunk per-head attn^T [D, QCH] bf16
        p_exp = pool("p_exp", 20)     # exp(score) tiles bf16 (a full head + next)
        p_rt = pool("p_rt", 1)        # rope temps bf16
        p_acc = pool("p_acc", 2)      # softmax-sum accumulators bf16
        p_rbc = pool("p_rbc", 2)      # broadcast reciprocal f32
        p_ost = pool("p_ost", 4)      # output staging bf16
        p_pp = pool("p_pp", 6, "PSUM")    # kps/qps/st/pw/sum/bc/transpose ring
        p_av = pool("p_av", 2, "PSUM")    # attn-v accumulators (+proj V)

        wq_sb = p_w.tile([128, KT, FS], BF16, name="wq_sb")
        wk_sb = p_w.tile([128, KT, D], BF16, name="wk_sb")
        wv_sb = p_w.tile([128, KT, D], BF16, name="wv_sb")
        wo_sb = p_w.tile([128, QH, HID], BF16, name="wo_sb")
        mask_sb = p_w.tile([128, QCH], F32, name="mask_sb")
        onesq_sb = p_w.tile([128, 128], BF16, name="onesq_sb")
        eye_sb = p_w.tile([128, 128], BF16, name="eye_sb")
        cos_sb = p_w.tile([D, B, S], BF16, name="cos_sb")
        sin_sb = p_w.tile([D, B, S], BF16, name="sin_sb")
        # Startup order (scalar queue): tiny consts + chunk-0 trig slices, the
        # wq tiles the first sweep consumes, then wv (needed at sweep2), the
        # remaining trig slices, then wo (first outproj at ~150us).  wk and
        # the chunk-0 ht tiles ride the sync queue interleaved per-kt inside
        # the first sweep pass so the PE is fed from ~2us.
        nc.scalar.dma_start(out=mask_sb[:, :], in_=mask[:, :])
        nc.scalar.dma_start(out=eye_sb[:, :], in_=eye[:, :])
        nc.scalar.dma_start(out=cos_sb[:, 0, 0:QCH], in_=trig[0, 0][:, 0:QCH])
        nc.scalar.dma_start(out=sin_sb[:, 0, 0:QCH], in_=trig[0, 1][:, 0:QCH])
        for kt in range(KT):
            nc.scalar.dma_start(out=wq_sb[:, kt, :], in_=wq[kt])
        nc.scalar.dma_start(out=onesq_sb[:, :], in_=onesq[:, :])
        for kt in range(KT):
            nc.scalar.dma_start(out=wv_sb[:, kt, :], in_=wv[kt])
        for b in range(B):
            for tci in range(TC):
                if b == 0 and tci == 0:
                    continue
                sl = slice(tci * QCH, (tci + 1) * QCH)
                nc.scalar.dma_start(out=cos_sb[:, b, sl], in_=trig[b, 0][:, sl])
                nc.scalar.dma_start(out=sin_sb[:, b, sl], in_=trig[b, 1][:, sl])
        for f in range(QH):
            nc.scalar.dma_start(out=wo_sb[:, f, :], in_=wo[f])

        def rope_evict(b, dst, ps, tci):
            # dst[:, :] = ps * cos + rotate_half(ps) * sin  (write bf16)
            # bf16 temporaries: DVE runs 2 elems/cycle for 16-bit vs 1 for f32.
            sl = slice(tci * QCH, (tci + 1) * QCH)
            cs = cos_sb[:, b, sl]
            sn = sin_sb[:, b, sl]
            t1 = p_rt.tile([128, QCH], BF16, name="rt1", tag="rt1")
            t2 = p_rt.tile([128, QCH], BF16, name="rt2", tag="rt2")
            nc.vector.tensor_mul(t1[:, :], ps[:, :], cs)
            nc.vector.tensor_mul(t2[0:HALF, :], ps[HALF:D, :], sn[0:HALF, :])
            nc.vector.tensor_mul(t2[HALF:D, :], ps[0:HALF, :], sn[HALF:D, :])
            nc.vector.tensor_sub(dst[0:HALF, :], t1[0:HALF, :], t2[0:HALF, :])
            nc.vector.tensor_add(dst[HALF:D, :], t1[HALF:D, :], t2[HALF:D, :])

        # Output projection emitters for one chunk (32 (tt,hc) groups); popped
        # one-per-kt into later streams once all four at-tiles are applied.
        def emit_op_group(g):
            bb, tt, tt4, hc, atp = g
            csl = slice(tt4 * 128, (tt4 + 1) * 128)
            pw = p_pp.tile([128, QCH], F32, name="ps_wo", tag="pp")
            for f in range(QH):
                nc.tensor.matmul(pw[:, :], atp[f][:, csl],
                                 wo_sb[:, f, hc * QCH:(hc + 1) * QCH],
                                 start=(f == 0), stop=(f == QH - 1))
            o = p_ost.tile([128, QCH], BF16, name="o_t", tag="ost")
            # evictions mostly on vector: each o-copy on the scalar queue sits
            # ahead of later exps and stalls the score psum ring, so scalar
            # (which must stay clear for the exp stream) takes only every 4th.
            # gpsimd cannot read PSUM, so it can't help here.
            if hc % 4 == 3:
                nc.scalar.copy(o[:, :], pw[:, :])
            else:
                nc.vector.tensor_copy(o[:, :], pw[:, :])
            nc.sync.dma_start(out=out[bb, tt, hc], in_=o[:, :])

        def make_op_groups(bb, tcip, atp):
            return [(bb, tcip * TPC + tt4, tt4, hc, atp)
                    for tt4 in range(TPC) for hc in range(HC)]

        # ---- norm pipeline + deferred-AV state (crosses streams/chunks) ----
        # P1: (acc, av, at_dst) awaiting sum+bcast in the current stream.
        # P2: (rb, av, at_dst) awaiting apply at the next stream's slot 0.
        # prev_av: the last finished head's AV work, flushed one-per-slot into
        #          the next stream (or the next chunk's projection sweep).
        st_ = {"P1": None, "P2": None, "prev_av": None}

        def emit_sumbc(P1):
            # fused softmax sum + partition-broadcast: an all-ones stationary
            # makes every psum row m equal sum_k acc[k, q] in ONE matmul
            # (replaces sum matmul + sbuf roundtrip + K=1 broadcast matmul).
            acc, av, at_dst = P1
            pb = p_pp.tile([128, QCH], F32, name="ps_pb", tag="pp")
            nc.tensor.matmul(pb[:, :], onesq_sb[:, :], acc[:, :],
                             start=True, stop=True)
            rb = p_rbc.tile([128, QCH], F32, name="rb", tag="rb")
            nc.vector.reciprocal_approx_fast(rb[:, :], pb[:, :])
            return (rb, av, at_dst)

        def emit_apply(P2):
            rb, av, at_dst = P2
            nc.vector.tensor_mul(at_dst[:, :], av[:, :], rb[:, :])

        def emit_av_step(pa, kt):
            e, lo, n = pa["es"][kt]
            nc.tensor.matmul(pa["av"][:, lo:QCH], pa["vb"][kt][:, :], e[:, 0:n],
                             start=(kt == 0), stop=(kt == pa["nk"] - 1))

        def stream_slot(kt, oq):
            # Fixed-slot work interleaved into a score stream: deferred AV
            # one-per-kt, norm stages at slots 1/3, outproj pops from kt>=3.
            # (The pending apply runs at stream END — right after the AV chain
            # it reads stops, and before any later psum alloc can rebind it.)
            pa = st_["prev_av"]
            if pa is not None and kt < pa["nk"]:
                emit_av_step(pa, kt)
                if kt == pa["nk"] - 1:
                    st_["prev_av"] = None
            if kt == 1 and st_["P1"] is not None:
                st_["P2"] = emit_sumbc(st_["P1"])
                st_["P1"] = None
            if kt >= 3 and oq:
                emit_op_group(oq.pop(0))

        def sweep1_slot(g):
            # Slot work inside the 5x32 projection sweep: the previous
            # chunk's last-head AV flush (g 0..nk'-1), then its norm stages
            # late enough that kps' psum slot has been freed by its RoPE
            # eviction (sum@g40, bc@g46) and its apply once its AV chain
            # stopped (g52).  No outproj pops: the pp ring is full of qps.
            pa = st_["prev_av"]
            if pa is not None and g < pa["nk"]:
                emit_av_step(pa, g)
                if g == pa["nk"] - 1:
                    st_["prev_av"] = None
            if g == 40 and st_["P1"] is not None:
                st_["P2"] = emit_sumbc(st_["P1"])
                st_["P1"] = None
            if g == 52 and st_["P2"] is not None:
                emit_apply(st_["P2"])
                st_["P2"] = None

        prev_op = None    # (b, tci, at_c) of the chunk awaiting outproj

        for b in range(B):
            vb = []       # V tiles [128 tok, D], global k-tile index
            ktb = p_kt.tile([D, S], BF16, name="ktb", tag="kt")
            for tci in range(TC):
                first = (b == 0 and tci == 0)
                oq = make_op_groups(*prev_op) if prev_op else []
                ktb_c = ktb[:, tci * QCH:(tci + 1) * QCH]
                qtb_c = [p_qt.tile([D, QCH], BF16, name=f"qtc{f}", tag="qt")
                         for f in range(QH)]
                # ---- sweep 1: five passes (K, q0..q3) over the kt tiles;
                #      each pass's RoPE eviction trails it on the DVE ----
                hts = []
                for p in range(1 + QH):
                    ps = p_pp.tile([128, QCH], F32, name=f"ps_p{p}", tag="pp")
                    for kt in range(KT):
                        if p == 0:
                            if first:
                                nc.sync.dma_start(out=wk_sb[:, kt, :], in_=wk[kt])
                            t = p_ht.tile([128, QCH], BF16, name="ht_t", tag="ht")
                            nc.sync.dma_start(out=t[:, :], in_=ht[b, kt, tci])
                            hts.append(t)
                        w_sl = (wk_sb[:, kt, :] if p == 0 else
                                wq_sb[:, kt, (p - 1) * D:p * D])
                        nc.tensor.matmul(ps[:, :], w_sl, hts[kt][:, :],
                                         start=(kt == 0), stop=(kt == KT - 1))
                        sweep1_slot(p * KT + kt)
                    if p == 0:
                        rope_evict(b, ktb_c, ps, tci)
                    else:
                        rope_evict(b, qtb_c[p - 1], ps, tci)
                # ---- sweep 2: V^T accumulation, then PE-transpose ----
                vps = p_av.tile([128, QCH], F32, name="ps_v", tag="av")
                for kt in range(KT):
                    nc.tensor.matmul(vps[:, :], wv_sb[:, kt, :], hts[kt][:, :],
                                     start=(kt == 0), stop=(kt == KT - 1))
                vtt = p_vtt.tile([128, QCH], BF16, name="vtt", tag="vtt")
                nc.vector.tensor_copy(vtt[:, :], vps[:, :])
                for sub in range(TPC):
                    pt = p_pp.tile([128, 128], BF16, name="ps_vt", tag="pp")
                    nc.tensor.transpose(pt[:, :], vtt[:, sub * 128:(sub + 1) * 128],
                                        eye_sb[:, :])
                    v = p_vt.tile([128, D], BF16, name="v_t", tag="vt")
                    nc.scalar.copy(v[:, :], pt[:, :])
                    vb.append(v)

                # ---- attention streams: scores for head h interleaved with
                #      the deferred AV of head h-1, norm slots, outproj pops ----
                at_c = [p_at.tile([D, QCH], BF16, name=f"atc{f}", tag="at")
                        for f in range(QH)]
                nk = KPQ * (tci + 1)
                for h in range(QH):
                    av = p_av.tile([128, QCH], F32, name="ps_av", tag="av")
                    acc = p_acc.tile([128, QCH], BF16, name="acc", tag="acc")
                    es = [None] * nk
                    for kt in range(nk):
                        j = kt - KPQ * tci          # >=0 inside diagonal band
                        lo = max(j, 0) * 128        # first live column
                        n = QCH - lo
                        st = p_pp.tile([128, QCH], F32, name="ps_st", tag="pp")
                        nc.tensor.matmul(
                            st[:, 0:n], ktb[:, kt * 128:(kt + 1) * 128],
                            qtb_c[h][:, lo:QCH], start=True, stop=True)
                        if j >= 0:  # diagonal tile: triangular mask on 1st 128
                            nc.vector.tensor_add(st[:, 0:128], st[:, 0:128],
                                                 mask_sb[:, 0:128])
                        e = p_exp.tile([128, QCH], BF16, name="e_t", tag="exp")
                        nc.scalar.activation(e[:, 0:n], st[:, 0:n], EXP, scale=SC)
                        # softmax-sum accumulation stays on the DVE: gpsimd
                        # runs these [128,512] adds ~2.5x slower and the
                        # serial acc chain then stalls the sum matmuls.
                        if kt == 0:
                            nc.vector.tensor_copy(acc[:, :], e[:, :])
                        else:
                            nc.vector.tensor_add(acc[:, lo:QCH], acc[:, lo:QCH],
                                                 e[:, 0:n])
                        es[kt] = (e, lo, n)
                        stream_slot(kt, oq)
                    assert st_["prev_av"] is None
                    assert st_["P1"] is None
                    # apply the head-before-last now: its AV chain stopped in
                    # this stream, and deferring further would let a later
                    # p_av allocation rebind its psum slot.
                    if st_["P2"] is not None:
                        emit_apply(st_["P2"])
                        st_["P2"] = None
                    st_["prev_av"] = {"av": av, "es": es, "vb": list(vb), "nk": nk}
                    st_["P1"] = (acc, av, at_c[h])
                # leftover outproj groups of the previous chunk
                for g in oq:
                    emit_op_group(g)
                prev_op = (b, tci, at_c)

        # ---- tail: flush the last head's AV + normalization + final outproj ----
        if st_["P2"] is not None:
            emit_apply(st_["P2"])   # at of head 2 of the last chunk
            st_["P2"] = None
        pa = st_["prev_av"]
        for kt in range(pa["nk"]):
            emit_av_step(pa, kt)
            if kt == 1 and st_["P1"] is not None:
                st_["P2"] = emit_sumbc(st_["P1"])
                st_["P1"] = None
        st_["prev_av"] = None
        emit_apply(st_["P2"])       # at of head 3
        st_["P2"] = None
        for g in make_op_groups(*prev_op):
            emit_op_group(g)
    nc.finalize()  # Bacc: runs compile() (reg alloc, wait splitting) + freeze
    return nc


def shard_inputs(hidden_states, cos, sin, Wq, Wk, Wv, Wo, n_cores=N_CORES,
                 QH=4, D=128, QCH=512):
    """Host-side prep: transpose/tile/bf16-round everything per core."""
    bf16 = ml_dtypes.bfloat16
    B, S, HID = hidden_states.shape
    FS = QH * D
    KT = HID // 128
    TC = S // QCH

    hT = hidden_states.astype(bf16).transpose(0, 2, 1)           # [B, HID, S]
    ht_t = np.ascontiguousarray(
        hT.reshape(B, KT, 128, TC, QCH).transpose(0, 1, 3, 2, 4))
    trig = np.ascontiguousarray(np.stack(
        [cos.transpose(0, 2, 1), sin.transpose(0, 2, 1)], axis=1)
    ).astype(bf16)

    kk = np.arange(128)[:, None]
    cc = np.arange(QCH)[None, :]
    maskv = np.where(cc < kk, np.float32(NEG), np.float32(0.0))
    onesqv = np.ones((128, 128), dtype=bf16)
    eyev = np.eye(128, dtype=bf16)

    in_maps = []
    for i in range(n_cores):
        wq_i = Wq[i * FS:(i + 1) * FS, :].T.astype(bf16).reshape(KT, 128, FS)
        wk_i = Wk[i * D:(i + 1) * D, :].T.astype(bf16).reshape(KT, 128, D)
        wv_i = Wv[i * D:(i + 1) * D, :].T.astype(bf16).reshape(KT, 128, D)
        wo_i = Wo[:, i * FS:(i + 1) * FS].T.astype(bf16).reshape(QH, D, HID)
        in_maps.append(dict(ht=ht_t, trig=trig, wq=wq_i, wk=wk_i, wv=wv_i,
                            wo=wo_i, mask=maskv, onesq=onesqv, eye=eyev))
    return in_maps


_NC_CACHE = {}


def kernel(hidden_states, cos, sin, Wq, Wk, Wv, Wo, _trace=False):
    hidden_states = np.asarray(hidden_states)
    cos = np.asarray(cos)
    sin = np.asarray(sin)
    Wq, Wk, Wv, Wo = (np.asarray(a) for a in (Wq, Wk, Wv, Wo))
    B, S, HID = hidden_states.shape

    key = (B, S, HID)
    nc = _NC_CACHE.get(key)
    if nc is None:
        nc = _NC_CACHE[key] = build_core_kernel(B, S, HID)

    in_maps = shard_inputs(hidden_states, cos, sin, Wq, Wk, Wv, Wo)
    res = run_bass_kernel_spmd(nc, in_maps, core_ids=list(range(N_CORES)),
                               trace=_trace)
    kernel._last_results = res

    acc = res.results[0]["out"].astype(np.float32)
    for r in res.results[1:]:
        acc += r["out"].astype(np.float32)
    # [B, TT, HC, 128, QCH] -> [B, S, HID]
    TT = S // 128
    HC = HID // 512
    full = acc.transpose(0, 1, 3, 2, 4).reshape(B, S, HID)
    return np.ascontiguousarray(full)


# revision 32
# speedup vs baseline: 1.1953x; 1.0842x over previous
# Trainium2 Bass kernel: Llama-style attention block (GQA + RoPE + causal),
# tensor-parallel across heads on 8 NeuronCores.
#
# Full-shape contract: kernel(**inputs) takes the unsharded numpy inputs and
# returns the full [B, S, HID] float32 output.
#
# Sharding strategy (per core i of 8):
#   - 4 query heads (rows i*512:(i+1)*512 of Wq) + 1 kv head (rows i*128.. of Wk/Wv)
#   - Wo is sharded row-wise (its columns i*512:(i+1)*512); each core emits a
#     partial [B,S,HID] product (bf16) which the host sums at gather time.
# All weights/activations are pre-transposed and pre-tiled on the host so the
# device kernel needs zero on-chip transposes of activations:
#   matmul(out[M,N], lhsT[K,M], rhs[K,N]) contracts over the partition dim K.
#
# Schedule (deferred-AV): the AV matmuls for head h are interleaved one-per-kt
# into head h+1's score stream, so by the time an AV issues, its exp input was
# produced a full head earlier and the PE never waits on the scalar engine.
# The softmax normalization is a 3-stage pipeline riding fixed slots of the
# next stream: sum(h) @ stream h+1 slot 1, bcast+recip(h) @ slot 3, apply(h)
# at stream h+1's end (right after av(h)'s chain stops).  The last head of
# each chunk flushes its AV + norm work into the next chunk's projection sweep.
#
# The projection sweep runs as 5 sequential passes (K, q0..q3), each followed
# immediately by its RoPE eviction on the DVE: psum banks free progressively
# during the sweep (instead of all five at the end), so the next stream's
# score tiles never wait on the psum ring, and all RoPE work overlaps the
# sweep itself instead of stalling the boundary.
#
# Startup DMA is just-in-time: wk/ht interleaved per-kt on the sync queue,
# chunk-0 cos/sin slices + wq early on the scalar queue, wv before the bulk
# trig/wo loads.

import os
import sys
from contextlib import ExitStack

for _p in ("/opt/trn_rl_repo", "/root/.axon_site/_ro/trn_rl_repo"):
    if os.path.isdir(_p) and _p not in sys.path:
        sys.path.append(_p)

import ml_dtypes
import numpy as np

import concourse.bass as bass
import concourse.mybir as mybir
import concourse.tile as tile
from concourse import bacc
from concourse.bass_utils import run_bass_kernel_spmd

BF16 = mybir.dt.bfloat16
F32 = mybir.dt.float32
NEG = -1.0e9
N_CORES = 8


def build_core_kernel(B, S, HID, QH=4, D=128, QCH=512):
    """SPMD per-core program. QH query heads + 1 kv head per core.

    DRAM parameter layouts (host pre-tiles everything):
      ht   [B, KT, TC, 128, QCH]  bf16  hidden^T tiles
      trig [B, 2, D, S]           bf16  cos^T / sin^T
      wq   [KT, 128, QH*D]        bf16  Wq_core^T tiles
      wk   [KT, 128, D]           bf16
      wv   [KT, 128, D]           bf16
      wo   [QH, D, HID]           bf16  Wo_core^T rows blocked per head
      mask [128, QCH]             f32   additive causal triangle
      ones [128, 1]               bf16  partition-sum lhsT
      onesr [1, 128]              bf16  broadcast lhsT
      eye  [128, 128]             bf16
      out  [B, TT, HC, 128, QCH]  bf16  partial output tiles
    """
    FS = QH * D          # per-core feature slice of the qkv/attn space
    KT = HID // 128      # contraction tiles for projections
    TC = S // QCH        # 512-token chunks
    TT = S // 128        # 128-token tiles
    TPC = QCH // 128     # token tiles per chunk
    HC = HID // QCH      # output hid chunks
    KPQ = QCH // 128     # k-tiles per q-chunk (diagonal band width)
    HALF = D // 2
    SC = float(1.0 / np.sqrt(D))
    EXP = mybir.ActivationFunctionType.Exp

    # Bacc (not plain Bass): its compile pipeline splits multi-sem waits into
    # EventSemaphore instructions — the DMA DIRECT2D struct has one wait slot.
    KP = KT // 2         # kt pairs: DRAM layouts pair adjacent kt tiles so
    HC2 = HC // 2        # each DMA partition-line is 2KB (the DMA engines are
                         # packet-rate bound at ~85ns/descriptor, so 1KB lines
                         # cap ht streaming at ~190 GB/s)
    nc = bacc.Bacc(None)
    ht = nc.declare_dram_parameter("ht", [B, KP, TC, 128, 2, QCH], BF16, isOutput=False)
    trig = nc.declare_dram_parameter("trig", [B, 2, D, S], BF16, isOutput=False)
    wq = nc.declare_dram_parameter("wq", [KP, 128, 2, FS], BF16, isOutput=False)
    wk = nc.declare_dram_parameter("wk", [KP, 128, 2, D], BF16, isOutput=False)
    wv = nc.declare_dram_parameter("wv", [KP, 128, 2, D], BF16, isOutput=False)
    wo = nc.declare_dram_parameter("wo", [QH, D, HID], BF16, isOutput=False)
    mask = nc.declare_dram_parameter("mask", [128, QCH], F32, isOutput=False)
    onesq = nc.declare_dram_parameter("onesq", [128, 128], BF16, isOutput=False)
    eye = nc.declare_dram_parameter("eye", [128, 128], BF16, isOutput=False)
    out = nc.declare_dram_parameter("out", [B, TT, HC2, 128, 2, QCH], BF16, isOutput=True)

    with ExitStack() as ctx:
        tc = ctx.enter_context(tile.TileContext(nc))
        pool = lambda name, bufs, space=None: ctx.enter_context(
            tc.tile_pool(name=name, bufs=bufs, **({"space": space} if space else {}))
        )
        p_w = pool("p_w", 1)          # weights + constants, loaded once
        p_ht = pool("p_ht", 22)       # streamed hidden^T pair-tiles [128,2,QCH]
        p_qt = pool("p_qt", 8)        # per-chunk per-head Q^T [D, QCH] bf16
        p_kt = pool("p_kt", 1)        # K^T [D, S] bf16 (persistent per batch)
        p_vtt = pool("p_vtt", 3)      # V^T staging before transpose
        p_vt = pool("p_vt", S // 128 + 4)  # V tiles [128 tok, D]
        p_at = pool("p_at", 8)        # per-chunk per-head attn^T [D, QCH] bf16
        p_exp = pool("p_exp", 20)     # exp(score) tiles bf16 (a full head + next)
        p_rt = pool("p_rt", 1)        # rope temps bf16
        p_acc = pool("p_acc", 2)      # softmax-sum accumulators bf16
        p_rbc = pool("p_rbc", 2)      # broadcast reciprocal f32
        p_ost = pool("p_ost", 4)      # output staging bf16
        p_pp = pool("p_pp", 6, "PSUM")    # kps/qps/st/pw/sum/bc/transpose ring
        p_av = pool("p_av", 2, "PSUM")    # attn-v accumulators (+proj V)

        wq_sb = p_w.tile([128, KT, FS], BF16, name="wq_sb")
        wk_sb = p_w.tile([128, KT, D], BF16, name="wk_sb")
        wv_sb = p_w.tile([128, KT, D], BF16, name="wv_sb")
        wo_sb = p_w.tile([128, QH, HID], BF16, name="wo_sb")
        mask_sb = p_w.tile([128, QCH], F32, name="mask_sb")
        onesq_sb = p_w.tile([128, 128], BF16, name="onesq_sb")
        eye_sb = p_w.tile([128, 128], BF16, name="eye_sb")
        cos_sb = p_w.tile([D, B, S], BF16, name="cos_sb")
        sin_sb = p_w.tile([D, B, S], BF16, name="sin_sb")
        # Startup order (scalar queue): tiny consts + chunk-0 trig slices, the
        # wq tiles the first sweep consumes, then wv (needed at sweep2), the
        # remaining trig slices, then wo (first outproj at ~150us).  wk and
        # the chunk-0 ht tiles ride the sync queue interleaved per-kt inside
        # the first sweep pass so the PE is fed from ~2us.
        nc.scalar.dma_start(out=mask_sb[:, :], in_=mask[:, :])
        nc.scalar.dma_start(out=eye_sb[:, :], in_=eye[:, :])
        nc.scalar.dma_start(out=cos_sb[:, 0, 0:QCH], in_=trig[0, 0][:, 0:QCH])
        nc.scalar.dma_start(out=sin_sb[:, 0, 0:QCH], in_=trig[0, 1][:, 0:QCH])
        for kp in range(KP):
            nc.scalar.dma_start(out=wq_sb[:, 2 * kp:2 * kp + 2, :], in_=wq[kp])
        nc.scalar.dma_start(out=onesq_sb[:, :], in_=onesq[:, :])
        for kp in range(KP):
            nc.scalar.dma_start(out=wv_sb[:, 2 * kp:2 * kp + 2, :], in_=wv[kp])
        for b in range(B):
            for tci in range(TC):
                if b == 0 and tci == 0:
                    continue
                sl = slice(tci * QCH, (tci + 1) * QCH)
                nc.scalar.dma_start(out=cos_sb[:, b, sl], in_=trig[b, 0][:, sl])
                nc.scalar.dma_start(out=sin_sb[:, b, sl], in_=trig[b, 1][:, sl])
        for f in range(QH):
            nc.scalar.dma_start(out=wo_sb[:, f, :], in_=wo[f])

        def rope_evict(b, dst, ps, tci):
            # dst[:, :] = ps * cos + rotate_half(ps) * sin  (write bf16)
            # bf16 temporaries: DVE runs 2 elems/cycle for 16-bit vs 1 for f32.
            sl = slice(tci * QCH, (tci + 1) * QCH)
            cs = cos_sb[:, b, sl]
            sn = sin_sb[:, b, sl]
            t1 = p_rt.tile([128, QCH], BF16, name="rt1", tag="rt1")
            t2 = p_rt.tile([128, QCH], BF16, name="rt2", tag="rt2")
            nc.vector.tensor_mul(t1[:, :], ps[:, :], cs)
            nc.vector.tensor_mul(t2[0:HALF, :], ps[HALF:D, :], sn[0:HALF, :])
            nc.vector.tensor_mul(t2[HALF:D, :], ps[0:HALF, :], sn[HALF:D, :])
            nc.vector.tensor_sub(dst[0:HALF, :], t1[0:HALF, :], t2[0:HALF, :])
            nc.vector.tensor_add(dst[HALF:D, :], t1[HALF:D, :], t2[HALF:D, :])

        # Output projection emitters for one chunk (32 (tt,hc) groups); popped
        # one-per-kt into later streams once all four at-tiles are applied.
        ost_pend = {}

        def emit_op_group(g):
            bb, tt, tt4, hc, atp = g
            csl = slice(tt4 * 128, (tt4 + 1) * 128)
            pw = p_pp.tile([128, QCH], F32, name="ps_wo", tag="pp")
            for f in range(QH):
                nc.tensor.matmul(pw[:, :], atp[f][:, csl],
                                 wo_sb[:, f, hc * QCH:(hc + 1) * QCH],
                                 start=(f == 0), stop=(f == QH - 1))
            # hc pairs stage into one [128, 2, QCH] tile -> one DMA with 2KB
            # partition-lines (the DMA engines are descriptor-rate bound).
            if hc % 2 == 0:
                o = p_ost.tile([128, 2, QCH], BF16, name="o_t", tag="ost")
                ost_pend[0] = o
            else:
                o = ost_pend.pop(0)
            # evictions mostly on vector: each o-copy on the scalar queue sits
            # ahead of later exps and stalls the score psum ring, so scalar
            # (which must stay clear for the exp stream) takes only every 4th.
            # gpsimd cannot read PSUM, so it can't help here.
            if hc % 4 == 3:
                nc.scalar.copy(o[:, hc % 2, :], pw[:, :])
            else:
                nc.vector.tensor_copy(o[:, hc % 2, :], pw[:, :])
            if hc % 2 == 1:
                nc.sync.dma_start(out=out[bb, tt, hc // 2], in_=o[:, :, :])

        def make_op_groups(bb, tcip, atp):
            return [(bb, tcip * TPC + tt4, tt4, hc, atp)
                    for tt4 in range(TPC) for hc in range(HC)]

        # ---- norm pipeline + deferred-AV state (crosses streams/chunks) ----
        # P1: (acc, av, at_dst) awaiting sum+bcast in the current stream.
        # P2: (rb, av, at_dst) awaiting apply at the next stream's slot 0.
        # prev_av: the last finished head's AV work, flushed one-per-slot into
        #          the next stream (or the next chunk's projection sweep).
        st_ = {"P1": None, "P2": None, "prev_av": None}

        def emit_sumbc(P1):
            # fused softmax sum + partition-broadcast: an all-ones stationary
            # makes every psum row m equal sum_k acc[k, q] in ONE matmul
            # (replaces sum matmul + sbuf roundtrip + K=1 broadcast matmul).
            acc, av, at_dst = P1
            pb = p_pp.tile([128, QCH], F32, name="ps_pb", tag="pp")
            nc.tensor.matmul(pb[:, :], onesq_sb[:, :], acc[:, :],
                             start=True, stop=True)
            rb = p_rbc.tile([128, QCH], F32, name="rb", tag="rb")
            nc.vector.reciprocal_approx_fast(rb[:, :], pb[:, :])
            return (rb, av, at_dst)

        def emit_apply(P2):
            rb, av, at_dst = P2
            nc.vector.tensor_mul(at_dst[:, :], av[:, :], rb[:, :])

        def emit_av_step(pa, kt):
            e, lo, n = pa["es"][kt]
            nc.tensor.matmul(pa["av"][:, lo:QCH], pa["vb"][kt][:, :], e[:, 0:n],
                             start=(kt == 0), stop=(kt == pa["nk"] - 1))

        def stream_slot(kt, oq):
            # Fixed-slot work interleaved into a score stream: deferred AV
            # one-per-kt, norm stages at slots 1/3, outproj pops from kt>=3.
            # (The pending apply runs at stream END — right after the AV chain
            # it reads stops, and before any later psum alloc can rebind it.)
            pa = st_["prev_av"]
            if pa is not None and kt < pa["nk"]:
                emit_av_step(pa, kt)
                if kt == pa["nk"] - 1:
                    st_["prev_av"] = None
            if kt == 1 and st_["P1"] is not None:
                st_["P2"] = emit_sumbc(st_["P1"])
                st_["P1"] = None
            if kt >= 3 and oq:
                emit_op_group(oq.pop(0))

        def sweep1_slot(g):
            # Slot work inside the 5x32 projection sweep: the previous
            # chunk's last-head AV flush (g 0..nk'-1), then its norm stages
            # late enough that kps' psum slot has been freed by its RoPE
            # eviction (sum@g40, bc@g46) and its apply once its AV chain
            # stopped (g52).  No outproj pops: the pp ring is full of qps.
            pa = st_["prev_av"]
            if pa is not None and g < pa["nk"]:
                emit_av_step(pa, g)
                if g == pa["nk"] - 1:
                    st_["prev_av"] = None
            if g == 40 and st_["P1"] is not None:
                st_["P2"] = emit_sumbc(st_["P1"])
                st_["P1"] = None
            if g == 52 and st_["P2"] is not None:
                emit_apply(st_["P2"])
                st_["P2"] = None

        prev_op = None    # (b, tci, at_c) of the chunk awaiting outproj

        for b in range(B):
            vb = []       # V tiles [128 tok, D], global k-tile index
            ktb = p_kt.tile([D, S], BF16, name="ktb", tag="kt")
            for tci in range(TC):
                first = (b == 0 and tci == 0)
                oq = make_op_groups(*prev_op) if prev_op else []
                ktb_c = ktb[:, tci * QCH:(tci + 1) * QCH]
                qtb_c = [p_qt.tile([D, QCH], BF16, name=f"qtc{f}", tag="qt")
                         for f in range(QH)]
                # ---- sweep 1: five passes (K, q0..q3) over the kt tiles;
                #      each pass's RoPE eviction trails it on the DVE ----
                hts = []
                for p in range(1 + QH):
                    ps = p_pp.tile([128, QCH], F32, name=f"ps_p{p}", tag="pp")
                    for kt in range(KT):
                        kp, c = kt // 2, kt % 2
                        if p == 0 and c == 0:
                            if first:
                                nc.sync.dma_start(
                                    out=wk_sb[:, 2 * kp:2 * kp + 2, :], in_=wk[kp])
                            t = p_ht.tile([128, 2, QCH], BF16, name="ht_t", tag="ht")
                            nc.sync.dma_start(out=t[:, :, :], in_=ht[b, kp, tci])
                            hts.append(t)
                        w_sl = (wk_sb[:, kt, :] if p == 0 else
                                wq_sb[:, kt, (p - 1) * D:p * D])
                        nc.tensor.matmul(ps[:, :], w_sl, hts[kp][:, c, :],
                                         start=(kt == 0), stop=(kt == KT - 1))
                        sweep1_slot(p * KT + kt)
                    if p == 0:
                        rope_evict(b, ktb_c, ps, tci)
                    else:
                        rope_evict(b, qtb_c[p - 1], ps, tci)
                # ---- sweep 2: V^T accumulation, then PE-transpose ----
                vps = p_av.tile([128, QCH], F32, name="ps_v", tag="av")
                for kt in range(KT):
                    nc.tensor.matmul(vps[:, :], wv_sb[:, kt, :],
                                     hts[kt // 2][:, kt % 2, :],
                                     start=(kt == 0), stop=(kt == KT - 1))
                vtt = p_vtt.tile([128, QCH], BF16, name="vtt", tag="vtt")
                nc.vector.tensor_copy(vtt[:, :], vps[:, :])
                for sub in range(TPC):
                    pt = p_pp.tile([128, 128], BF16, name="ps_vt", tag="pp")
                    nc.tensor.transpose(pt[:, :], vtt[:, sub * 128:(sub + 1) * 128],
                                        eye_sb[:, :])
                    v = p_vt.tile([128, D], BF16, name="v_t", tag="vt")
                    nc.scalar.copy(v[:, :], pt[:, :])
                    vb.append(v)

                # ---- attention streams: scores for head h interleaved with
                #      the deferred AV of head h-1, norm slots, outproj pops ----
                at_c = [p_at.tile([D, QCH], BF16, name=f"atc{f}", tag="at")
                        for f in range(QH)]
                nk = KPQ * (tci + 1)
                for h in range(QH):
                    av = p_av.tile([128, QCH], F32, name="ps_av", tag="av")
                    acc = p_acc.tile([128, QCH], BF16, name="acc", tag="acc")
                    es = [None] * nk
                    for kt in range(nk):
                        j = kt - KPQ * tci          # >=0 inside diagonal band
                        lo = max(j, 0) * 128        # first live column
                        n = QCH - lo
                        st = p_pp.tile([128, QCH], F32, name="ps_st", tag="pp")
                        nc.tensor.matmul(
                            st[:, 0:n], ktb[:, kt * 128:(kt + 1) * 128],
                            qtb_c[h][:, lo:QCH], start=True, stop=True)
                        if j >= 0:  # diagonal tile: triangular mask on 1st 128
                            nc.vector.tensor_add(st[:, 0:128], st[:, 0:128],
                                                 mask_sb[:, 0:128])
                        e = p_exp.tile([128, QCH], BF16, name="e_t", tag="exp")
                        nc.scalar.activation(e[:, 0:n], st[:, 0:n], EXP, scale=SC)
                        # softmax-sum accumulation stays on the DVE: gpsimd
                        # runs these [128,512] adds ~2.5x slower and the
                        # serial acc chain then stalls the sum matmuls.
                        if kt == 0:
                            nc.vector.tensor_copy(acc[:, :], e[:, :])
                        else:
                            nc.vector.tensor_add(acc[:, lo:QCH], acc[:, lo:QCH],
                                                 e[:, 0:n])
                        es[kt] = (e, lo, n)
                        stream_slot(kt, oq)
                    assert st_["prev_av"] is None
                    assert st_["P1"] is None
                    # apply the head-before-last now: its AV chain stopped in
                    # this stream, and deferring further would let a later
                    # p_av allocation rebind its psum slot.
                    if st_["P2"] is not None:
                        emit_apply(st_["P2"])
                        st_["P2"] = None
                    st_["prev_av"] = {"av": av, "es": es, "vb": list(vb), "nk": nk}
                    st_["P1"] = (acc, av, at_c[h])
                # leftover outproj groups of the previous chunk
                for g in oq:
                    emit_op_group(g)
                prev_op = (b, tci, at_c)

        # ---- tail: flush the last head's AV + normalization + final outproj ----
        if st_["P2"] is not None:
            emit_apply(st_["P2"])   # at of head 2 of the last chunk
            st_["P2"] = None
        pa = st_["prev_av"]
        for kt in range(pa["nk"]):
            emit_av_step(pa, kt)
            if kt == 1 and st_["P1"] is not None:
                st_["P2"] = emit_sumbc(st_["P1"])
                st_["P1"] = None
        st_["prev_av"] = None
        emit_apply(st_["P2"])       # at of head 3
        st_["P2"] = None
        for g in make_op_groups(*prev_op):
            emit_op_group(g)
    nc.finalize()  # Bacc: runs compile() (reg alloc, wait splitting) + freeze
    return nc


def shard_inputs(hidden_states, cos, sin, Wq, Wk, Wv, Wo, n_cores=N_CORES,
                 QH=4, D=128, QCH=512):
    """Host-side prep: transpose/tile/bf16-round everything per core."""
    bf16 = ml_dtypes.bfloat16
    B, S, HID = hidden_states.shape
    FS = QH * D
    KT = HID // 128
    TC = S // QCH

    KP = KT // 2
    hT = hidden_states.astype(bf16).transpose(0, 2, 1)           # [B, HID, S]
    # pair-tile layout [B, KP, TC, 128, 2, QCH]: DMA lines of 2KB/partition
    ht_t = np.ascontiguousarray(
        hT.reshape(B, KP, 2, 128, TC, QCH).transpose(0, 1, 4, 3, 2, 5))
    trig = np.ascontiguousarray(np.stack(
        [cos.transpose(0, 2, 1), sin.transpose(0, 2, 1)], axis=1)
    ).astype(bf16)

    kk = np.arange(128)[:, None]
    cc = np.arange(QCH)[None, :]
    maskv = np.where(cc < kk, np.float32(NEG), np.float32(0.0))
    onesqv = np.ones((128, 128), dtype=bf16)
    eyev = np.eye(128, dtype=bf16)

    in_maps = []
    for i in range(n_cores):
        wq_i = np.ascontiguousarray(
            Wq[i * FS:(i + 1) * FS, :].T.astype(bf16)
            .reshape(KP, 2, 128, FS).transpose(0, 2, 1, 3))
        wk_i = np.ascontiguousarray(
            Wk[i * D:(i + 1) * D, :].T.astype(bf16)
            .reshape(KP, 2, 128, D).transpose(0, 2, 1, 3))
        wv_i = np.ascontiguousarray(
            Wv[i * D:(i + 1) * D, :].T.astype(bf16)
            .reshape(KP, 2, 128, D).transpose(0, 2, 1, 3))
        wo_i = Wo[:, i * FS:(i + 1) * FS].T.astype(bf16).reshape(QH, D, HID)
        in_maps.append(dict(ht=ht_t, trig=trig, wq=wq_i, wk=wk_i, wv=wv_i,
                            wo=wo_i, mask=maskv, onesq=onesqv, eye=eyev))
    return in_maps


_NC_CACHE = {}


def kernel(hidden_states, cos, sin, Wq, Wk, Wv, Wo, _trace=False):
    hidden_states = np.asarray(hidden_states)
    cos = np.asarray(cos)
    sin = np.asarray(sin)
    Wq, Wk, Wv, Wo = (np.asarray(a) for a in (Wq, Wk, Wv, Wo))
    B, S, HID = hidden_states.shape

    key = (B, S, HID)
    nc = _NC_CACHE.get(key)
    if nc is None:
        nc = _NC_CACHE[key] = build_core_kernel(B, S, HID)

    in_maps = shard_inputs(hidden_states, cos, sin, Wq, Wk, Wv, Wo)
    res = run_bass_kernel_spmd(nc, in_maps, core_ids=list(range(N_CORES)),
                               trace=_trace)
    kernel._last_results = res

    acc = res.results[0]["out"].astype(np.float32)
    for r in res.results[1:]:
        acc += r["out"].astype(np.float32)
    # [B, TT, HC2, 128, 2, QCH] -> [B, S, HID]
    full = acc.transpose(0, 1, 3, 2, 4, 5).reshape(B, S, HID)
    return np.ascontiguousarray(full)
